# revision 9
# baseline (speedup 1.0000x reference)
"""Discounted-return scan + normalize, distributed over 8 TRN2 NeuronCores.

Problem: y_i = r_i + 0.99*y_{i+1} (suffix scan over T=2**25 rewards), then
(y - mean) / (std + eps).

Strategy:
  - Host reverses rewards so the device runs a plain forward scan
    (state = g*state + s_t) along the SBUF free dimension via the
    TensorTensorScanArith ISA op on the Vector engine.
  - The T axis is sharded 8 ways; each core's shard is further split across
    128 SBUF partitions. Instead of propagating carries between partitions /
    cores, every partition scans a W-element "burn-in" prefix (overlapping
    reads): with g=0.99, a carry's influence after W=4096 steps is ~1e-18
    relative — far below f32 resolution, so results match the exact scan.
  - mean/std: per-chunk accumulated sum (GpSimd) and sum-of-squares
    (ScalarE Square+accum) feed one tiny 8-core AllGather; every core then
    redundantly computes the global scale/shift and normalizes in place.
"""

import os
import sys

import numpy as np

for _p in ("/opt/trn_rl_repo", "/root/.axon_site/_ro/trn_rl_repo"):
    if os.path.isdir(_p) and _p not in sys.path:
        sys.path.insert(0, _p)

DISCOUNT = 0.99
EPS = 0.0001
T = 33554432  # 2**25
N_CORES = 8
P = 128  # SBUF partitions


def _build_nc(C, W, F):
    """Build the per-core Bass graph.

    C: elements per core (excluding burn-in), multiple of P*F
    W: burn-in prefix length per partition (also the first chunk size)
    F: scan/DMA chunk size (columns)
    """
    import concourse.bacc as bacc
    import concourse.bass as bass
    import concourse.mybir as mybir
    from concourse import tile

    fp32 = mybir.dt.float32
    Alu = mybir.AluOpType
    Act = mybir.ActivationFunctionType
    Axis = mybir.AxisListType

    L = C // P  # valid columns per partition
    assert L % F == 0
    NV = L // F  # number of valid chunks
    # chunk list: (start_col, width); chunk 0 is the burn-in
    chunks = [(0, W)] + [(W + i * F, F) for i in range(NV)]

    nc = bacc.Bacc(
        "TRN2",
        target_bir_lowering=False,
        debug=False,
        enable_asserts=True,
        num_devices=N_CORES,
    )

    s_ext = nc.dram_tensor("s", [C + W], fp32, kind="ExternalInput")
    out_ext = nc.dram_tensor("out", [C], fp32, kind="ExternalOutput")
    cc_in = nc.dram_tensor("cc_in", [1, 2], fp32)
    cc_out = nc.dram_tensor("cc_out", [N_CORES, 2], fp32)

    inv_T = 1.0 / float(C * N_CORES)

    with tile.TileContext(nc) as tc:
        with (
            tc.tile_pool(name="main", bufs=1) as main,
            tc.tile_pool(name="small", bufs=1) as small,
        ):
            resident = main.tile([P, W + L], fp32)
            scratch_sq = main.tile([P, F], fp32)
            scratch_sum = main.tile([P, F], fp32)

            g_tile = small.tile([P, 1], fp32)
            scol = small.tile([P, NV], fp32)
            qcol = small.tile([P, NV], fp32)
            s_vec = small.tile([P, 1], fp32)
            q_vec = small.tile([P, 1], fp32)
            cc_sb = small.tile([1, 2], fp32)
            gath = small.tile([P, 2 * N_CORES], fp32)
            tot_s = small.tile([P, 1], fp32)
            tot_q = small.tile([P, 1], fp32)
            mean = small.tile([P, 1], fp32)
            msq = small.tile([P, 1], fp32)
            var = small.tile([P, 1], fp32)
            std = small.tile([P, 1], fp32)
            inv = small.tile([P, 1], fp32)
            shift = small.tile([P, 1], fp32)

            nc.vector.memset(g_tile[:, :], DISCOUNT)

            # ---- DMA in: one strided transfer per chunk (rows overlap by W) ----
            for c0, cw in chunks:
                src = bass.AP(s_ext, c0, [[L, P], [1, cw]])
                nc.sync.dma_start(resident[:, c0 : c0 + cw], src)

            # ---- chained scans + per-chunk stats ----
            for t, (c0, cw) in enumerate(chunks):
                dst = resident[:, c0 : c0 + cw]
                initial = 0.0 if t == 0 else resident[:, c0 - 1 : c0]
                nc.vector.tensor_tensor_scan(
                    dst,
                    g_tile[:, 0:1].broadcast_to((P, cw)),
                    dst,
                    initial,
                    Alu.mult,
                    Alu.add,
                )
                if t >= 1:
                    i = t - 1
                    nc.scalar.activation(
                        scratch_sq[:, :cw],
                        dst,
                        Act.Square,
                        accum_out=qcol[:, i : i + 1],
                    )
                    nc.scalar.activation(
                        scratch_sum[:, :cw],
                        dst,
                        Act.Copy,
                        accum_out=scol[:, i : i + 1],
                    )

            # ---- local totals -> [1,2] -> AllGather -> [8,2] ----
            nc.vector.tensor_reduce(s_vec[:, :], scol[:, :], Axis.X, Alu.add)
            nc.vector.tensor_reduce(q_vec[:, :], qcol[:, :], Axis.X, Alu.add)
            nc.gpsimd.tensor_reduce(cc_sb[0:1, 0:1], s_vec[:, :], Axis.C, Alu.add)
            nc.gpsimd.tensor_reduce(cc_sb[0:1, 1:2], q_vec[:, :], Axis.C, Alu.add)
            nc.sync.dma_start(cc_in.ap(), cc_sb[0:1, :])
            nc.gpsimd.collective_compute(
                "AllGather",
                Alu.bypass,
                replica_groups=[list(range(N_CORES))],
                ins=[cc_in.ap().opt()],
                outs=[cc_out.ap().opt()],
            )
            # broadcast the 16 gathered floats to every partition
            gsrc = bass.AP(cc_out, 0, [[0, P], [1, 2 * N_CORES]])
            nc.sync.dma_start(gath[:, :], gsrc)

            # ---- global stats (every partition, redundantly) ----
            nc.vector.tensor_reduce(
                tot_s[:, :], gath[:, 0 : 2 * N_CORES : 2], Axis.X, Alu.add
            )
            nc.vector.tensor_reduce(
                tot_q[:, :], gath[:, 1 : 2 * N_CORES : 2], Axis.X, Alu.add
            )
            nc.vector.tensor_scalar(mean[:, :], tot_s[:, :], inv_T, None, Alu.mult)
            nc.vector.tensor_scalar(msq[:, :], tot_q[:, :], inv_T, None, Alu.mult)
            # var = msq - mean^2
            nc.vector.tensor_tensor(var[:, :], mean[:, :], mean[:, :], Alu.mult)
            nc.vector.tensor_tensor(var[:, :], msq[:, :], var[:, :], Alu.subtract)
            nc.scalar.activation(std[:, :], var[:, :], Act.Sqrt)
            nc.vector.tensor_scalar(std[:, :], std[:, :], EPS, None, Alu.add)
            nc.vector.reciprocal(inv[:, :], std[:, :])
            nc.vector.tensor_tensor(shift[:, :], mean[:, :], inv[:, :], Alu.mult)
            nc.vector.tensor_scalar(shift[:, :], shift[:, :], -1.0, None, Alu.mult)

            # ---- normalize in place + DMA out ----
            for i in range(NV):
                c0 = W + i * F
                seg = resident[:, c0 : c0 + F]
                nc.vector.tensor_scalar(
                    seg, seg, inv[:, 0:1], shift[:, 0:1], Alu.mult, Alu.add
                )
                dst = bass.AP(out_ext, i * F, [[L, P], [1, F]])
                nc.sync.dma_start(dst, seg)

    nc.compile()
    return nc


_CACHED = {}


def _get_nc(C, W, F):
    key = (C, W, F)
    if key not in _CACHED:
        _CACHED[key] = _build_nc(C, W, F)
    return _CACHED[key]


def run_sharded(rewards, C=None, W=4096, F=4096, **spmd_kwargs):
    """Shard, run on 8 cores, gather. Returns (output, BassKernelResults)."""
    from concourse import bass_utils

    r = np.ascontiguousarray(np.asarray(rewards, dtype=np.float32))
    total = r.shape[0]
    if C is None:
        C = total // N_CORES
    assert C * N_CORES == total

    nc = _get_nc(C, W, F)

    s_pad = np.empty(total + W, dtype=np.float32)
    s_pad[:W] = 0.0
    s_pad[W:] = r[::-1]
    in_maps = [
        {"s": np.ascontiguousarray(s_pad[c * C : (c + 1) * C + W])}
        for c in range(N_CORES)
    ]
    res = bass_utils.run_bass_kernel_spmd(
        nc, in_maps, core_ids=list(range(N_CORES)), **spmd_kwargs
    )
    y = np.concatenate([res.results[c]["out"].reshape(-1) for c in range(N_CORES)])
    return np.ascontiguousarray(y[::-1]), res


def kernel(rewards):
    out, _ = run_sharded(rewards)
    return out


# revision 10
# speedup vs baseline: 1.0356x; 1.0356x over previous
"""Discounted-return scan + normalize, distributed over 8 TRN2 NeuronCores.

Problem: y_i = r_i + 0.99*y_{i+1} (suffix scan over T=2**25 rewards), then
(y - mean) / (std + eps).

Strategy:
  - Host reverses rewards so the device runs a plain forward scan
    (state = g*state + s_t) along the SBUF free dimension via the
    TensorTensorScanArith ISA op on the Vector engine.
  - The T axis is sharded 8 ways; each core's shard is further split across
    128 SBUF partitions. Instead of propagating carries between partitions /
    cores, every partition scans a W-element "burn-in" prefix (overlapping
    reads): with g=0.99, a carry's influence after W=2048 steps is ~1e-9
    relative — far below the f32 resolution of the result, so the output
    matches the exact scan.
  - mean/std: per-chunk accumulated sum / sum-of-squares (ScalarE
    activation accumulate; last chunk's sum on DVE) feed one tiny 8-core
    AllGather; every core then redundantly computes the global scale/shift
    and normalizes in place. A dummy AllGather early in the kernel absorbs
    the collective firmware cold-start while the scan chain runs.
"""

import os
import sys

import numpy as np

for _p in ("/opt/trn_rl_repo", "/root/.axon_site/_ro/trn_rl_repo"):
    if os.path.isdir(_p) and _p not in sys.path:
        sys.path.insert(0, _p)

DISCOUNT = 0.99
EPS = 0.0001
T = 33554432  # 2**25
N_CORES = 8
P = 128  # SBUF partitions


def _build_nc(C, W, F):
    """Build the per-core Bass graph.

    C: elements per core (excluding burn-in), multiple of P*F
    W: burn-in prefix length per partition (also the first chunk size)
    F: main scan chunk size (columns)
    """
    import concourse.bacc as bacc
    import concourse.bass as bass
    import concourse.mybir as mybir
    from concourse import tile

    fp32 = mybir.dt.float32
    Alu = mybir.AluOpType
    Act = mybir.ActivationFunctionType
    Axis = mybir.AxisListType

    L = C // P  # valid columns per partition
    R = L + W  # total row length
    assert L % F == 0

    # scan chunks: burn-in, then F-sized, with the last two halved to
    # shorten the post-scan stats tail on the critical path.
    scan_chunks = [(0, W)]
    c = W
    while c < R:
        if R - c == F:
            scan_chunks += [(c, F // 2), (c + F // 2, F // 2)]
            c = R
        else:
            scan_chunks.append((c, F))
            c += F
    valid_chunks = scan_chunks[1:]
    NV = len(valid_chunks)

    # DMA-in chunks: small first (gates the first scan), large after.
    dma_chunks = [(0, W)]
    c = W
    while c < R:
        cw = min(2 * F, R - c)
        dma_chunks.append((c, cw))
        c += cw

    nc = bacc.Bacc(
        "TRN2",
        target_bir_lowering=False,
        debug=False,
        enable_asserts=True,
        num_devices=N_CORES,
    )

    s_ext = nc.dram_tensor("s", [C + W], fp32, kind="ExternalInput")
    out_ext = nc.dram_tensor("out", [C], fp32, kind="ExternalOutput")
    cc_in = nc.dram_tensor("cc_in", [1, 2], fp32)
    cc_out = nc.dram_tensor("cc_out", [N_CORES, 2], fp32)
    warm_in = nc.dram_tensor("warm_in", [1, 2], fp32)
    warm_out = nc.dram_tensor("warm_out", [N_CORES, 2], fp32)

    inv_T = 1.0 / float(C * N_CORES)
    rg = [list(range(N_CORES))]

    with tile.TileContext(nc) as tc:
        with (
            tc.tile_pool(name="main", bufs=1) as main,
            tc.tile_pool(name="small", bufs=1) as small,
        ):
            resident = main.tile([P, R], fp32)
            scratch_sq = main.tile([P, F], fp32)
            scratch_sum = main.tile([P, F], fp32)
            scratch_dve = main.tile([P, F // 2], fp32)

            g_tile = small.tile([P, 1], fp32)
            warm_sb = small.tile([P, 2], fp32)
            scol = small.tile([P, NV], fp32)
            qcol = small.tile([P, NV], fp32)
            s_vec = small.tile([P, 1], fp32)
            q_vec = small.tile([P, 1], fp32)
            cc_sb = small.tile([1, 2], fp32)
            gath = small.tile([P, 2 * N_CORES], fp32)
            tot_s = small.tile([P, 1], fp32)
            tot_q = small.tile([P, 1], fp32)
            mean = small.tile([P, 1], fp32)
            msq = small.tile([P, 1], fp32)
            var = small.tile([P, 1], fp32)
            std = small.tile([P, 1], fp32)
            inv = small.tile([P, 1], fp32)
            shiftp = small.tile([P, 1], fp32)

            # ---- DMA in (alternating HWDGE rings) + collective warmup ----
            for k, (c0, cw) in enumerate(dma_chunks):
                src = bass.AP(s_ext, c0, [[L, P], [1, cw]])
                eng = nc.sync if k % 2 == 0 else nc.scalar
                eng.dma_start(resident[:, c0 : c0 + cw], src)

            nc.vector.memset(g_tile[:, :], DISCOUNT)
            nc.vector.memset(warm_sb[:, :], 0.0)
            # warm the CC firmware path while the scan chain runs
            nc.sync.dma_start(warm_in.ap(), warm_sb[0:1, :])
            nc.gpsimd.collective_compute(
                "AllGather",
                Alu.bypass,
                replica_groups=rg,
                ins=[warm_in.ap().opt()],
                outs=[warm_out.ap().opt()],
            )
            # load the sqrt activation table before it's on the critical path
            nc.scalar.activation(warm_sb[:, 0:1], g_tile[:, 0:1], Act.Sqrt)

            # ---- chained scans + per-chunk stats ----
            last = len(scan_chunks) - 1
            for t, (c0, cw) in enumerate(scan_chunks):
                dst = resident[:, c0 : c0 + cw]
                initial = 0.0 if t == 0 else resident[:, c0 - 1 : c0]
                nc.vector.tensor_tensor_scan(
                    dst,
                    g_tile[:, 0:1].broadcast_to((P, cw)),
                    dst,
                    initial,
                    Alu.mult,
                    Alu.add,
                )
                if t >= 1:
                    i = t - 1
                    nc.scalar.activation(
                        scratch_sq[:, :cw],
                        dst,
                        Act.Square,
                        accum_out=qcol[:, i : i + 1],
                    )
                    if t == last:
                        # keep the tail short: last chunk's sum on DVE,
                        # parallel with ScalarE's Square.
                        nc.vector.tensor_scalar(
                            scratch_dve[:, :cw],
                            dst,
                            1.0,
                            None,
                            Alu.mult,
                            Alu.add,
                            accum_out=scol[:, i : i + 1],
                        )
                    else:
                        nc.scalar.activation(
                            scratch_sum[:, :cw],
                            dst,
                            Act.Copy,
                            accum_out=scol[:, i : i + 1],
                        )

            # ---- local totals -> [1,2] -> AllGather -> [8,2] ----
            nc.vector.tensor_reduce(s_vec[:, :], scol[:, :], Axis.X, Alu.add)
            nc.vector.tensor_reduce(q_vec[:, :], qcol[:, :], Axis.X, Alu.add)
            nc.gpsimd.tensor_reduce(cc_sb[0:1, 0:1], s_vec[:, :], Axis.C, Alu.add)
            nc.gpsimd.tensor_reduce(cc_sb[0:1, 1:2], q_vec[:, :], Axis.C, Alu.add)
            nc.sync.dma_start(cc_in.ap(), cc_sb[0:1, :])
            nc.gpsimd.collective_compute(
                "AllGather",
                Alu.bypass,
                replica_groups=rg,
                ins=[cc_in.ap().opt()],
                outs=[cc_out.ap().opt()],
            )
            # broadcast the 16 gathered floats to every partition
            gsrc = bass.AP(cc_out, 0, [[0, P], [1, 2 * N_CORES]])
            nc.sync.dma_start(gath[:, :], gsrc)

            # ---- global stats (every partition, redundantly) ----
            nc.vector.tensor_reduce(
                tot_s[:, :], gath[:, 0 : 2 * N_CORES : 2], Axis.X, Alu.add
            )
            nc.vector.tensor_reduce(
                tot_q[:, :], gath[:, 1 : 2 * N_CORES : 2], Axis.X, Alu.add
            )
            nc.vector.tensor_scalar(mean[:, :], tot_s[:, :], inv_T, None, Alu.mult)
            nc.vector.tensor_scalar(msq[:, :], tot_q[:, :], inv_T, None, Alu.mult)
            nc.vector.tensor_tensor(var[:, :], mean[:, :], mean[:, :], Alu.mult)
            nc.vector.tensor_tensor(var[:, :], msq[:, :], var[:, :], Alu.subtract)
            nc.scalar.activation(std[:, :], var[:, :], Act.Sqrt)
            nc.vector.tensor_scalar(std[:, :], std[:, :], EPS, None, Alu.add)
            nc.vector.reciprocal(inv[:, :], std[:, :])
            nc.vector.tensor_tensor(shiftp[:, :], mean[:, :], inv[:, :], Alu.mult)

            # ---- normalize in place + paired DMA out (alternating rings) ----
            pend = None  # (start_col, width) of normalized-but-unsent region
            k = 0
            for t, (c0, cw) in enumerate(valid_chunks):
                seg = resident[:, c0 : c0 + cw]
                # out = y*inv - mean*inv
                nc.vector.tensor_scalar(
                    seg, seg, inv[:, 0:1], shiftp[:, 0:1], Alu.mult, Alu.subtract
                )
                if pend is None and t < len(valid_chunks) - 1:
                    pend = (c0, cw)
                    continue
                o0, ow = (pend[0], pend[1] + cw) if pend else (c0, cw)
                pend = None
                dst = bass.AP(out_ext, o0 - W, [[L, P], [1, ow]])
                eng = nc.sync if k % 2 == 0 else nc.scalar
                k += 1
                eng.dma_start(dst, resident[:, o0 : o0 + ow])

    nc.compile()
    return nc


_CACHED = {}


def _get_nc(C, W, F):
    key = (C, W, F)
    if key not in _CACHED:
        _CACHED[key] = _build_nc(C, W, F)
    return _CACHED[key]


def run_sharded(rewards, C=None, W=2048, F=4096, **spmd_kwargs):
    """Shard, run on 8 cores, gather. Returns (output, BassKernelResults)."""
    from concourse import bass_utils

    r = np.ascontiguousarray(np.asarray(rewards, dtype=np.float32))
    total = r.shape[0]
    if C is None:
        C = total // N_CORES
    assert C * N_CORES == total

    nc = _get_nc(C, W, F)

    s_pad = np.empty(total + W, dtype=np.float32)
    s_pad[:W] = 0.0
    s_pad[W:] = r[::-1]
    in_maps = [
        {"s": np.ascontiguousarray(s_pad[c * C : (c + 1) * C + W])}
        for c in range(N_CORES)
    ]
    res = bass_utils.run_bass_kernel_spmd(
        nc, in_maps, core_ids=list(range(N_CORES)), **spmd_kwargs
    )
    y = np.concatenate([res.results[c]["out"].reshape(-1) for c in range(N_CORES)])
    return np.ascontiguousarray(y[::-1]), res


def kernel(rewards):
    out, _ = run_sharded(rewards)
    return out


# revision 17
# speedup vs baseline: 1.0875x; 1.0501x over previous
"""Discounted-return scan + normalize, distributed over 8 TRN2 NeuronCores.

Problem: y_i = r_i + 0.99*y_{i+1} (suffix scan over T=2**25 rewards), then
(y - mean) / (std + eps).

Strategy:
  - Host reverses rewards so the device runs a plain forward scan
    (state = g*state + s_t) along the SBUF free dimension via the
    TensorTensorScanArith ISA op on the Vector engine.
  - The T axis is sharded 8 ways; each core's shard is further split across
    128 SBUF partitions. Instead of propagating carries between partitions /
    cores, every partition scans a W-element "burn-in" prefix (overlapping
    reads): with g=0.99, a carry's influence after W=2048 steps is ~1e-9
    relative — far below the f32 resolution of the result, so the output
    matches the exact scan.
  - mean/std: per-chunk accumulated sum / sum-of-squares (ScalarE
    activation accumulate; last chunk's sum on DVE) feed one tiny 8-core
    AllGather; every core then redundantly computes the global scale/shift
    and normalizes in place. A dummy AllGather early in the kernel absorbs
    the collective firmware cold-start while the scan chain runs.
"""

import os
import sys

import numpy as np

for _p in ("/opt/trn_rl_repo", "/root/.axon_site/_ro/trn_rl_repo"):
    if os.path.isdir(_p) and _p not in sys.path:
        sys.path.insert(0, _p)

DISCOUNT = 0.99
EPS = 0.0001
T = 33554432  # 2**25
N_CORES = 8
P = 128  # SBUF partitions


def _build_nc(C, W, F):
    """Build the per-core Bass graph.

    C: elements per core (excluding burn-in), multiple of P*F
    W: burn-in prefix length per partition (also the first chunk size)
    F: main scan chunk size (columns)
    """
    import concourse.bacc as bacc
    import concourse.bass as bass
    import concourse.mybir as mybir
    from concourse import tile

    fp32 = mybir.dt.float32
    Alu = mybir.AluOpType
    Act = mybir.ActivationFunctionType
    Axis = mybir.AxisListType

    L = C // P  # valid columns per partition
    R = L + W  # total row length
    assert L % F == 0

    # chunk widths: two small burn-in chunks so the scan chain starts as
    # soon as the first small DMA lands, F-sized steady state, last two
    # halved to shorten the post-scan stats tail. DMA-in uses the same
    # chunking, all on ONE HWDGE ring so completions arrive in order.
    widths = [W // 2, W - W // 2]
    rem = R - W
    while rem > F:
        widths.append(F)
        rem -= F
    widths += [rem // 2, rem - rem // 2]
    scan_chunks = []
    c = 0
    for w in widths:
        scan_chunks.append((c, w))
        c += w
    assert c == R
    n_burn = 2  # first two chunks are burn-in
    valid_chunks = scan_chunks[n_burn:]
    NV = len(valid_chunks)
    dma_chunks = scan_chunks

    nc = bacc.Bacc(
        "TRN2",
        target_bir_lowering=False,
        debug=False,
        enable_asserts=True,
        num_devices=N_CORES,
    )

    s_ext = nc.dram_tensor("s", [C + W], fp32, kind="ExternalInput")
    out_ext = nc.dram_tensor("out", [C], fp32, kind="ExternalOutput")
    cc_in = nc.dram_tensor("cc_in", [1, 2], fp32)
    cc_out = nc.dram_tensor("cc_out", [N_CORES, 2], fp32)
    warm_in = nc.dram_tensor("warm_in", [1, 2], fp32)
    warm_out = nc.dram_tensor("warm_out", [N_CORES, 2], fp32)

    inv_T = 1.0 / float(C * N_CORES)
    rg = [list(range(N_CORES))]

    with tile.TileContext(nc) as tc:
        with (
            tc.tile_pool(name="main", bufs=1) as main,
            tc.tile_pool(name="small", bufs=1) as small,
        ):
            resident = main.tile([P, R], fp32)
            scratch_sq = main.tile([P, F], fp32)
            scratch_sum = main.tile([P, F], fp32)
            scratch_dve = main.tile([P, F // 2], fp32)

            g_tile = small.tile([P, 1], fp32)
            warm_sb = small.tile([P, 2], fp32)
            scol = small.tile([P, NV], fp32)
            qcol = small.tile([P, NV], fp32)
            s_vec = small.tile([P, 1], fp32)
            q_vec = small.tile([P, 1], fp32)
            cc_sb = small.tile([1, 2], fp32)
            gath = small.tile([P, 2 * N_CORES], fp32)
            tot_s = small.tile([P, 1], fp32)
            tot_q = small.tile([P, 1], fp32)
            mean = small.tile([P, 1], fp32)
            msq = small.tile([P, 1], fp32)
            var = small.tile([P, 1], fp32)
            std = small.tile([P, 1], fp32)
            inv = small.tile([P, 1], fp32)
            shiftp = small.tile([P, 1], fp32)

            nc.vector.memset(g_tile[:, :], DISCOUNT)
            nc.vector.memset(warm_sb[:, :], 0.0)
            # warm the CC firmware path while the scan chain runs; its input
            # DMA goes via the GpSimd SWDGE queue so the big in-DMAs on the
            # sync HWDGE ring can't delay it.
            nc.gpsimd.dma_start(warm_in.ap(), warm_sb[0:1, :])
            nc.gpsimd.collective_compute(
                "AllGather",
                Alu.bypass,
                replica_groups=rg,
                ins=[warm_in.ap().opt()],
                outs=[warm_out.ap().opt()],
            )
            # load the sqrt activation table before it's on the critical path
            nc.scalar.activation(std[:, :], g_tile[:, 0:1], Act.Sqrt)

            # ---- DMA in: one ring, in order, so completions are sequential ----
            for c0, cw in dma_chunks:
                src = bass.AP(s_ext, c0, [[L, P], [1, cw]])
                nc.sync.dma_start(resident[:, c0 : c0 + cw], src)

            # ---- chained scans + per-chunk stats ----
            last = len(scan_chunks) - 1
            for t, (c0, cw) in enumerate(scan_chunks):
                dst = resident[:, c0 : c0 + cw]
                initial = 0.0 if t == 0 else resident[:, c0 - 1 : c0]
                nc.vector.tensor_tensor_scan(
                    dst,
                    g_tile[:, 0:1].broadcast_to((P, cw)),
                    dst,
                    initial,
                    Alu.mult,
                    Alu.add,
                )
                if t >= n_burn:
                    i = t - n_burn
                    nc.scalar.activation(
                        scratch_sq[:, :cw],
                        dst,
                        Act.Square,
                        accum_out=qcol[:, i : i + 1],
                    )
                    if t == last:
                        # keep the tail short: last chunk's sum on DVE,
                        # parallel with ScalarE's Square.
                        nc.vector.tensor_scalar(
                            scratch_dve[:, :cw],
                            dst,
                            1.0,
                            None,
                            Alu.mult,
                            Alu.add,
                            accum_out=scol[:, i : i + 1],
                        )
                    else:
                        nc.scalar.activation(
                            scratch_sum[:, :cw],
                            dst,
                            Act.Copy,
                            accum_out=scol[:, i : i + 1],
                        )

            # ---- local totals -> [1,2] -> AllGather -> [8,2] ----
            nc.vector.tensor_reduce(s_vec[:, :], scol[:, :], Axis.X, Alu.add)
            nc.vector.tensor_reduce(q_vec[:, :], qcol[:, :], Axis.X, Alu.add)
            nc.gpsimd.tensor_reduce(cc_sb[0:1, 0:1], s_vec[:, :], Axis.C, Alu.add)
            nc.gpsimd.tensor_reduce(cc_sb[0:1, 1:2], q_vec[:, :], Axis.C, Alu.add)
            nc.scalar.dma_start(cc_in.ap(), cc_sb[0:1, :])
            nc.gpsimd.collective_compute(
                "AllGather",
                Alu.bypass,
                replica_groups=rg,
                ins=[cc_in.ap().opt()],
                outs=[cc_out.ap().opt()],
            )
            # broadcast the 16 gathered floats to every partition
            gsrc = bass.AP(cc_out, 0, [[0, P], [1, 2 * N_CORES]])
            nc.scalar.dma_start(gath[:, :], gsrc)

            # ---- global stats (every partition, redundantly) ----
            nc.vector.tensor_reduce(
                tot_s[:, :], gath[:, 0 : 2 * N_CORES : 2], Axis.X, Alu.add
            )
            nc.vector.tensor_reduce(
                tot_q[:, :], gath[:, 1 : 2 * N_CORES : 2], Axis.X, Alu.add
            )
            nc.vector.tensor_scalar(mean[:, :], tot_s[:, :], inv_T, None, Alu.mult)
            nc.vector.tensor_scalar(msq[:, :], tot_q[:, :], inv_T, None, Alu.mult)
            nc.vector.tensor_tensor(var[:, :], mean[:, :], mean[:, :], Alu.mult)
            nc.vector.tensor_tensor(var[:, :], msq[:, :], var[:, :], Alu.subtract)
            nc.scalar.activation(std[:, :], var[:, :], Act.Sqrt)
            nc.vector.tensor_scalar(std[:, :], std[:, :], EPS, None, Alu.add)
            nc.vector.reciprocal(inv[:, :], std[:, :])
            nc.vector.tensor_tensor(shiftp[:, :], mean[:, :], inv[:, :], Alu.mult)

            # ---- normalize in place + paired DMA out (alternating rings) ----
            pend = None  # (start_col, width) of normalized-but-unsent region
            k = 0
            for t, (c0, cw) in enumerate(valid_chunks):
                seg = resident[:, c0 : c0 + cw]
                # out = y*inv - mean*inv
                nc.vector.tensor_scalar(
                    seg, seg, inv[:, 0:1], shiftp[:, 0:1], Alu.mult, Alu.subtract
                )
                if pend is None and t < len(valid_chunks) - 1:
                    pend = (c0, cw)
                    continue
                o0, ow = (pend[0], pend[1] + cw) if pend else (c0, cw)
                pend = None
                dst = bass.AP(out_ext, o0 - W, [[L, P], [1, ow]])
                k += 1
                nc.sync.dma_start(dst, resident[:, o0 : o0 + ow])

    nc.compile()
    return nc


_CACHED = {}


def _get_nc(C, W, F):
    key = (C, W, F)
    if key not in _CACHED:
        _CACHED[key] = _build_nc(C, W, F)
    return _CACHED[key]


def run_sharded(rewards, C=None, W=1024, F=4096, **spmd_kwargs):
    """Shard, run on 8 cores, gather. Returns (output, BassKernelResults)."""
    from concourse import bass_utils

    r = np.ascontiguousarray(np.asarray(rewards, dtype=np.float32))
    total = r.shape[0]
    if C is None:
        C = total // N_CORES
    assert C * N_CORES == total

    nc = _get_nc(C, W, F)

    s_pad = np.empty(total + W, dtype=np.float32)
    s_pad[:W] = 0.0
    s_pad[W:] = r[::-1]
    in_maps = [
        {"s": np.ascontiguousarray(s_pad[c * C : (c + 1) * C + W])}
        for c in range(N_CORES)
    ]
    res = bass_utils.run_bass_kernel_spmd(
        nc, in_maps, core_ids=list(range(N_CORES)), **spmd_kwargs
    )
    y = np.concatenate([res.results[c]["out"].reshape(-1) for c in range(N_CORES)])
    return np.ascontiguousarray(y[::-1]), res


def kernel(rewards):
    out, _ = run_sharded(rewards)
    return out


# revision 22
# speedup vs baseline: 1.1710x; 1.0768x over previous
"""Discounted-return scan + normalize, distributed over 8 TRN2 NeuronCores.

Problem: y_i = r_i + 0.99*y_{i+1} (suffix scan over T=2**25 rewards), then
(y - mean) / (std + eps).

Strategy:
  - Host reverses rewards so the device runs a plain forward scan
    (state = g*state + s_t) along the SBUF free dimension via the
    TensorTensorScanArith ISA op on the Vector engine.
  - The T axis is sharded 8 ways; each core's shard is further split across
    128 SBUF partitions. Instead of propagating carries between partitions /
    cores, every partition scans a W-element "burn-in" prefix (overlapping
    reads): with g=0.99, a carry's influence after W=2048 steps is ~1e-9
    relative — far below the f32 resolution of the result, so the output
    matches the exact scan.
  - mean/std: per-chunk accumulated sum / sum-of-squares (ScalarE
    activation accumulate; last chunk's sum on DVE) feed one tiny 8-core
    AllGather; every core then redundantly computes the global scale/shift
    and normalizes in place. A dummy AllGather early in the kernel absorbs
    the collective firmware cold-start while the scan chain runs.
"""

import os
import sys

import numpy as np

for _p in ("/opt/trn_rl_repo", "/root/.axon_site/_ro/trn_rl_repo"):
    if os.path.isdir(_p) and _p not in sys.path:
        sys.path.insert(0, _p)

DISCOUNT = 0.99
EPS = 0.0001
T = 33554432  # 2**25
N_CORES = 8
P = 128  # SBUF partitions


def _build_nc(C, W, F):
    """Build the per-core Bass graph.

    C: elements per core (excluding burn-in), multiple of P*F
    W: burn-in prefix length per partition (also the first chunk size)
    F: main scan chunk size (columns)
    """
    import concourse.bacc as bacc
    import concourse.bass as bass
    import concourse.mybir as mybir
    from concourse import tile

    fp32 = mybir.dt.float32
    Alu = mybir.AluOpType
    Act = mybir.ActivationFunctionType
    Axis = mybir.AxisListType

    L = C // P  # valid columns per partition
    R = L + W  # total row length
    assert L % F == 0

    # chunk widths: a graduated ramp so the scan chain starts as soon as
    # the first small DMA lands and never outruns the (sequential) DMA
    # completions; F-sized steady state; last chunks halved to shorten the
    # post-scan stats tail. DMA-in uses the same chunking, all on ONE
    # HWDGE ring so completions arrive in order.
    widths = [W // 2, W - W // 2]
    rem = R - W
    ramp = W
    while ramp < F and rem - ramp >= F:
        widths.append(ramp)
        rem -= ramp
        ramp *= 2
    while rem > 2 * F:
        widths.append(F)
        rem -= F
    widths += [rem // 2, rem - rem // 2]
    scan_chunks = []
    c = 0
    for w in widths:
        scan_chunks.append((c, w))
        c += w
    assert c == R
    n_burn = 2  # first two chunks are burn-in
    valid_chunks = scan_chunks[n_burn:]
    NV = len(valid_chunks)
    dma_chunks = scan_chunks

    nc = bacc.Bacc(
        "TRN2",
        target_bir_lowering=False,
        debug=False,
        enable_asserts=True,
        num_devices=N_CORES,
    )

    s_ext = nc.dram_tensor("s", [C + W], fp32, kind="ExternalInput")
    out_ext = nc.dram_tensor("out", [C], fp32, kind="ExternalOutput")
    cc_in = nc.dram_tensor("cc_in", [1, 2], fp32)
    cc_out = nc.dram_tensor("cc_out", [N_CORES, 2], fp32)
    warm_in = nc.dram_tensor("warm_in", [1, 2], fp32)
    warm_out = nc.dram_tensor("warm_out", [N_CORES, 2], fp32)
    warm2_in = nc.dram_tensor("warm2_in", [1, 2], fp32)
    warm2_out = nc.dram_tensor("warm2_out", [N_CORES, 2], fp32)

    inv_T = 1.0 / float(C * N_CORES)
    rg = [list(range(N_CORES))]

    with tile.TileContext(nc) as tc:
        with (
            tc.tile_pool(name="main", bufs=1) as main,
            tc.tile_pool(name="small", bufs=1) as small,
        ):
            resident = main.tile([P, R], fp32)
            scratch_sq = main.tile([P, F], fp32)
            scratch_sum = main.tile([P, F], fp32)
            scratch_dve = main.tile([P, F], fp32)

            g_tile = small.tile([P, 1], fp32)
            warm_sb = small.tile([P, 2], fp32)
            scol = small.tile([P, NV], fp32)
            qcol = small.tile([P, NV], fp32)
            s_vec = small.tile([P, 1], fp32)
            q_vec = small.tile([P, 1], fp32)
            cc_sb = small.tile([1, 2], fp32)
            gath = small.tile([P, 2 * N_CORES], fp32)
            tot_s = small.tile([P, 1], fp32)
            tot_q = small.tile([P, 1], fp32)
            mean = small.tile([P, 1], fp32)
            msq = small.tile([P, 1], fp32)
            var = small.tile([P, 1], fp32)
            std = small.tile([P, 1], fp32)
            inv = small.tile([P, 1], fp32)
            shiftp = small.tile([P, 1], fp32)

            nc.vector.memset(g_tile[:, :], DISCOUNT)
            nc.vector.memset(warm_sb[:, :], 0.0)
            # warm the CC firmware path while the scan chain runs; its input
            # DMA goes via the GpSimd SWDGE queue so the big in-DMAs on the
            # sync HWDGE ring can't delay it.
            nc.gpsimd.dma_start(warm_in.ap(), warm_sb[0:1, :])
            nc.gpsimd.collective_compute(
                "AllGather",
                Alu.bypass,
                replica_groups=rg,
                ins=[warm_in.ap().opt()],
                outs=[warm_out.ap().opt()],
            )
            # load the sqrt activation table before it's on the critical path
            nc.scalar.activation(std[:, :], g_tile[:, 0:1], Act.Sqrt)

            # ---- DMA in: one ring, in order, so completions are sequential ----
            for c0, cw in dma_chunks:
                src = bass.AP(s_ext, c0, [[L, P], [1, cw]])
                nc.sync.dma_start(resident[:, c0 : c0 + cw], src)

            # ---- chained scans + per-chunk stats ----
            last = len(scan_chunks) - 1
            for t, (c0, cw) in enumerate(scan_chunks):
                dst = resident[:, c0 : c0 + cw]
                initial = 0.0 if t == 0 else resident[:, c0 - 1 : c0]
                nc.vector.tensor_tensor_scan(
                    dst,
                    g_tile[:, 0:1].broadcast_to((P, cw)),
                    dst,
                    initial,
                    Alu.mult,
                    Alu.add,
                )
                if t == len(scan_chunks) // 2:
                    # second CC warmup pinned mid-scan (ncfw settles between
                    # collectives; keeps the real one fast). Reading the
                    # previous chunk's scan output pins the doorbell here.
                    nc.gpsimd.dma_start(warm2_in.ap(), resident[0:1, c0 - 2 : c0])
                    nc.gpsimd.collective_compute(
                        "AllGather",
                        Alu.bypass,
                        replica_groups=rg,
                        ins=[warm2_in.ap().opt()],
                        outs=[warm2_out.ap().opt()],
                    )
                if t >= n_burn:
                    i = t - n_burn
                    nc.scalar.activation(
                        scratch_sq[:, :cw],
                        dst,
                        Act.Square,
                        accum_out=qcol[:, i : i + 1],
                    )
                    if t == last:
                        # keep the tail short: last chunk's sum on DVE,
                        # parallel with ScalarE's Square.
                        nc.vector.tensor_scalar(
                            scratch_dve[:, :cw],
                            dst,
                            1.0,
                            None,
                            Alu.mult,
                            Alu.add,
                            accum_out=scol[:, i : i + 1],
                        )
                    else:
                        nc.scalar.activation(
                            scratch_sum[:, :cw],
                            dst,
                            Act.Copy,
                            accum_out=scol[:, i : i + 1],
                        )

            # ---- local totals -> [1,2] -> AllGather -> [8,2] ----
            nc.vector.tensor_reduce(s_vec[:, :], scol[:, :], Axis.X, Alu.add)
            nc.vector.tensor_reduce(q_vec[:, :], qcol[:, :], Axis.X, Alu.add)
            nc.gpsimd.tensor_reduce(cc_sb[0:1, 0:1], s_vec[:, :], Axis.C, Alu.add)
            nc.gpsimd.tensor_reduce(cc_sb[0:1, 1:2], q_vec[:, :], Axis.C, Alu.add)
            nc.scalar.dma_start(cc_in.ap(), cc_sb[0:1, :])
            nc.gpsimd.collective_compute(
                "AllGather",
                Alu.bypass,
                replica_groups=rg,
                ins=[cc_in.ap().opt()],
                outs=[cc_out.ap().opt()],
            )
            # broadcast the 16 gathered floats to every partition
            gsrc = bass.AP(cc_out, 0, [[0, P], [1, 2 * N_CORES]])
            nc.scalar.dma_start(gath[:, :], gsrc)

            # ---- global stats (every partition, redundantly) ----
            nc.vector.tensor_reduce(
                tot_s[:, :], gath[:, 0 : 2 * N_CORES : 2], Axis.X, Alu.add
            )
            nc.vector.tensor_reduce(
                tot_q[:, :], gath[:, 1 : 2 * N_CORES : 2], Axis.X, Alu.add
            )
            nc.vector.tensor_scalar(mean[:, :], tot_s[:, :], inv_T, None, Alu.mult)
            nc.vector.tensor_scalar(msq[:, :], tot_q[:, :], inv_T, None, Alu.mult)
            nc.vector.tensor_tensor(var[:, :], mean[:, :], mean[:, :], Alu.mult)
            nc.vector.tensor_tensor(var[:, :], msq[:, :], var[:, :], Alu.subtract)
            nc.scalar.activation(std[:, :], var[:, :], Act.Sqrt)
            nc.vector.tensor_scalar(std[:, :], std[:, :], EPS, None, Alu.add)
            nc.vector.reciprocal(inv[:, :], std[:, :])
            nc.vector.tensor_tensor(shiftp[:, :], mean[:, :], inv[:, :], Alu.mult)

            # ---- normalize in place + paired DMA out (alternating rings) ----
            pend = None  # (start_col, width) of normalized-but-unsent region
            k = 0
            for t, (c0, cw) in enumerate(valid_chunks):
                seg = resident[:, c0 : c0 + cw]
                # out = y*inv - mean*inv
                nc.vector.tensor_scalar(
                    seg, seg, inv[:, 0:1], shiftp[:, 0:1], Alu.mult, Alu.subtract
                )
                if pend is None and t < len(valid_chunks) - 1:
                    pend = (c0, cw)
                    continue
                o0, ow = (pend[0], pend[1] + cw) if pend else (c0, cw)
                pend = None
                dst = bass.AP(out_ext, o0 - W, [[L, P], [1, ow]])
                k += 1
                nc.sync.dma_start(dst, resident[:, o0 : o0 + ow])

    nc.compile()
    return nc


_CACHED = {}


def _get_nc(C, W, F):
    key = (C, W, F)
    if key not in _CACHED:
        _CACHED[key] = _build_nc(C, W, F)
    return _CACHED[key]


def run_sharded(rewards, C=None, W=1024, F=4096, **spmd_kwargs):
    """Shard, run on 8 cores, gather. Returns (output, BassKernelResults)."""
    from concourse import bass_utils

    r = np.ascontiguousarray(np.asarray(rewards, dtype=np.float32))
    total = r.shape[0]
    if C is None:
        C = total // N_CORES
    assert C * N_CORES == total

    nc = _get_nc(C, W, F)

    s_pad = np.empty(total + W, dtype=np.float32)
    s_pad[:W] = 0.0
    s_pad[W:] = r[::-1]
    in_maps = [
        {"s": np.ascontiguousarray(s_pad[c * C : (c + 1) * C + W])}
        for c in range(N_CORES)
    ]
    res = bass_utils.run_bass_kernel_spmd(
        nc, in_maps, core_ids=list(range(N_CORES)), **spmd_kwargs
    )
    y = np.concatenate([res.results[c]["out"].reshape(-1) for c in range(N_CORES)])
    return np.ascontiguousarray(y[::-1]), res


def kernel(rewards):
    out, _ = run_sharded(rewards)
    return out
